# revision 17
# baseline (speedup 1.0000x reference)
"""Trainium2 Bass kernel for GQA attention (32 q heads / 16 kv heads, head_dim
128, L=2048, D=4608) with RoPE, tanh softcap 50, causal mask, o_proj.

Strategy: tensor-parallel over heads across 8 NeuronCores. Core c computes
q-heads 4c..4c+3 and kv-heads 2c..2c+1 end-to-end; the host sums the 8 partial
[L, D] outputs (bf16 partials, f32 host accumulation).

v2 design (vs the two-phase baseline):
  - single software-pipelined pass over the 4 q-chunks of 512: causality lets
    attention for chunk nq start right after its projections (K/V history for
    chunks <= nq is already computed), so the Scalar engine's tanh+exp stream
    (~200us) hides under the PE's projection matmuls instead of serializing a
    separate attention phase
  - PV computed in [d, q] layout (lhsT = V tile, rhs = P^T tile, 512-wide
    streams) so every PE matmul streams >= 256 columns and LDWEIGHTS stays
    shadow-loaded; this also eliminates the per-128-column PE transposes of
    the attention output (o_proj consumes [d, q] directly)
  - softmax denominator accumulated on the otherwise-idle GpSimd engine
    (tensor_add over P^T tiles + partition_all_reduce broadcast), reciprocal
    on DVE, folded into the PV psum drain multiply
  - rope drains moved off the Scalar engine: DVE multiplies read the
    projection psum directly (cos/sin mul + rotate-half add)
  - wq/wk/wv resident; wo streamed per (chunk, j) to fit SBUF; x staged per
    chunk; outputs written bf16
"""

import numpy as np
import ml_dtypes

import concourse.bass as bass
import concourse.mybir as mybir
import concourse.tile as tile
from concourse import bacc, bass_isa

F32 = mybir.dt.float32
BF16 = mybir.dt.bfloat16
BF16_NP = ml_dtypes.bfloat16
AF = mybir.ActivationFunctionType

N_HEADS = 32
N_KV = 16
HEAD_DIM = 128
ROPE_THETA = 10000.0
SOFTCAP = 50.0
SCALE = 1.0 / 12.0  # 1/sqrt(144)
L = 2048
D = 4608
N_CORES = 8
QH = N_HEADS // N_CORES        # 4 local q heads
KVH = N_KV // N_CORES          # 2 local kv heads
KC = D // 128                  # 36 contraction chunks
NQ = L // 512                  # 4 l-chunks of 512
LT = L // 128                  # 16 l-tiles of 128


DEBUG_TAPS = False


def _emit(nc):
    xt_d = nc.dram_tensor("xt", [2 * NQ, 128, KC * 256], BF16, kind="ExternalInput")
    wqt_d = nc.dram_tensor("wqt", [128, KC * QH * 128], BF16, kind="ExternalInput")
    wkt_d = nc.dram_tensor("wkt", [128, KC * KVH * 128], BF16, kind="ExternalInput")
    wvt_d = nc.dram_tensor("wvt", [128, KC * KVH * 128], BF16, kind="ExternalInput")
    wot_d = nc.dram_tensor("wot", [9, 128, QH * 512], BF16, kind="ExternalInput")
    cs_d = nc.dram_tensor("cs", [2 * NQ, 128, 512], BF16, kind="ExternalInput")
    masks_d = nc.dram_tensor("masks", [128, 1280], BF16, kind="ExternalInput")
    out_d = nc.dram_tensor("out", [NQ, 9, 128, 4 * 512], BF16, kind="ExternalOutput")
    if DEBUG_TAPS:
        qt_dbg = nc.dram_tensor("qt_dbg", [QH, 128, 512], BF16, kind="ExternalOutput")
        kt_dbg = nc.dram_tensor("kt_dbg", [KVH, 128, L], BF16, kind="ExternalOutput")
        ve_dbg = nc.dram_tensor("ve_dbg", [128, LT * 256], BF16, kind="ExternalOutput")
        at_dbg = nc.dram_tensor("at_dbg", [QH, 128, 512], BF16, kind="ExternalOutput")
        rb_dbg = nc.dram_tensor("rb_dbg", [QH, 128, 512], F32, kind="ExternalOutput")
        pt_dbg = nc.dram_tensor("pt_dbg", [4, 128, 512], BF16, kind="ExternalOutput")

    with tile.TileContext(nc) as tc:
        with (
            tc.tile_pool(name="const", bufs=1) as const,
            tc.tile_pool(name="wts", bufs=1) as wts,
            tc.tile_pool(name="wo", bufs=2) as wop,
            tc.tile_pool(name="xp", bufs=2) as xp,
            tc.tile_pool(name="cs", bufs=2) as csp,
            tc.tile_pool(name="qt", bufs=2) as qtp,
            tc.tile_pool(name="persist", bufs=1) as persist,
            tc.tile_pool(name="pt", bufs=1) as ptp,
            tc.tile_pool(name="rp", bufs=1) as rpp,
            tc.tile_pool(name="tt", bufs=1) as ttp,
            tc.tile_pool(name="dn", bufs=1) as dnp,
            tc.tile_pool(name="rb", bufs=1) as rbp,
            tc.tile_pool(name="at", bufs=3) as atp,
            tc.tile_pool(name="ob", bufs=2) as obp,
            tc.tile_pool(name="pj_psum", bufs=2, space="PSUM") as pj_psum,
            tc.tile_pool(name="sc_psum", bufs=2, space="PSUM") as sc_psum,
            tc.tile_pool(name="pv_psum", bufs=2, space="PSUM") as pv_psum,
            tc.tile_pool(name="op_psum", bufs=2, space="PSUM") as op_psum,
        ):
            # ---- persistent tensors ----
            KT = [persist.tile([128, L], BF16, tag=f"kt{g}", name=f"kt{g}")
                  for g in range(KVH)]
            VE = persist.tile([128, LT * 256], BF16, tag="ve", name="ve")
            QTS = [[None] * QH for _ in range(NQ)]
            # ---- prologue DMA: one 2D descriptor per tensor (descriptor
            # service pace is ~2us each regardless of size; count rules) ----
            def dma_x(s2):
                t = xp.tile([128, KC * 256], BF16, tag="x", name="xc")
                nc.sync.dma_start(t[:], xt_d[s2])
                return t

            xc0 = dma_x(0)
            wk = wts.tile([128, KC * KVH * 128], BF16, tag="wk", name="wk")
            nc.sync.dma_start(wk[:], wkt_d[:])
            wq = wts.tile([128, KC * QH * 128], BF16, tag="wq", name="wq")
            nc.sync.dma_start(wq[:], wqt_d[:])
            wv = wts.tile([128, KC * KVH * 128], BF16, tag="wv", name="wv")
            nc.sync.dma_start(wv[:], wvt_d[:])
            mtile = const.tile([128, 1280], BF16, tag="masks")
            nc.sync.dma_start(mtile[:], masks_d[:])
            moff = [0, 512, 896, 1152]
            maskt = [mtile[:, moff[o]:moff[o] + 512 - o * 128] for o in range(4)]

            def dma_cs(s2):
                t = csp.tile([128, 512], BF16, tag="cs", name="cs")
                nc.sync.dma_start(t[:], cs_d[s2])
                return t[:, 0:256], t[:, 256:512]

            x_next = [xc0]

            def rope_drain(ps, dst, cosc, sinc):
                """psum [128,256] f32 -> rotate-half rope -> dst bf16."""
                t1 = rpp.tile([128, 256], F32, tag="r1")
                nc.vector.tensor_mul(t1[:], ps[:], cosc[:])
                t2 = rpp.tile([128, 256], F32, tag="r2")
                nc.vector.tensor_mul(t2[0:64, :], ps[64:128, :], sinc[0:64, :])
                nc.vector.tensor_mul(t2[64:128, :], ps[0:64, :], sinc[64:128, :])
                nc.vector.tensor_add(dst[:], t1[:], t2[:])

            def proj_sub(s2):
                """Projections for 256-col sub-chunk s2 (K, Q, V + rope).

                Prefetches sub-chunk s2+1's x tiles (bufs=2 ring, no WAR
                wait) so projection matmuls never stall on staging DMA.
                """
                nq, half = s2 // 2, s2 % 2
                xc = x_next[0]
                if s2 + 1 < 2 * NQ:
                    x_next[0] = dma_x(s2 + 1)
                cosc, sinc = dma_cs(s2)
                cols = slice(half * 256, half * 256 + 256)
                for g in range(KVH):
                    ps = pj_psum.tile([128, 256], F32, tag="pj")
                    for k in range(KC):
                        nc.tensor.matmul(
                            ps[:], wk[:, k * 256 + g * 128:k * 256 + g * 128 + 128],
                            xc[:, k * 256:(k + 1) * 256],
                            start=(k == 0), stop=(k == KC - 1))
                    rope_drain(ps, KT[g][:, s2 * 256:(s2 + 1) * 256],
                               cosc, sinc)
                for h in range(QH):
                    if half == 0:
                        QTS[nq][h] = qtp.tile([128, 512], BF16, tag=f"qt{h}", name=f"qt{h}")
                    qt = QTS[nq][h]
                    ps = pj_psum.tile([128, 256], F32, tag="pj")
                    for k in range(KC):
                        nc.tensor.matmul(
                            ps[:], wq[:, k * 512 + h * 128:k * 512 + h * 128 + 128],
                            xc[:, k * 256:(k + 1) * 256],
                            start=(k == 0), stop=(k == KC - 1))
                    rope_drain(ps, qt[:, cols], cosc, sinc)
                for b in range(2):
                    mk = s2 * 2 + b
                    ps = pj_psum.tile([128, 256], F32, tag="pj")
                    for k in range(KC):
                        nc.tensor.matmul(
                            ps[:], xc[:, k * 256 + b * 128:k * 256 + b * 128 + 128],
                            wv[:, k * 256:(k + 1) * 256],
                            start=(k == 0), stop=(k == KC - 1))
                    nc.vector.tensor_copy(
                        VE[:, mk * 256:(mk + 1) * 256], ps[:])

            def proj_a(nq):
                proj_sub(2 * nq)

            def proj_b(nq):
                proj_sub(2 * nq + 1)

            def scores(nq, h):
                """scores -> tanh -> exp -> mask; GpSimd denom; rb recip."""
                g = h // 2
                nkt = 4 * nq + 4
                hp = h % 2
                pts = []
                dn = dnp.tile([128, 512], F32, tag="dn")
                for mk in range(nkt):
                    o = mk - 4 * nq
                    c0 = max(0, o) * 128
                    w = 512 - c0
                    ps_s = sc_psum.tile([128, 512], F32, tag="sc")
                    nc.tensor.matmul(
                        ps_s[:, 0:w], KT[g][:, mk * 128:(mk + 1) * 128],
                        QTS[nq][h][:, c0:512])
                    tt = ttp.tile([128, 512], F32, tag="tanh")
                    nc.scalar.activation(
                        tt[:, 0:w], ps_s[:, 0:w], AF.Tanh, scale=SCALE / SOFTCAP)
                    pt = ptp.tile([128, 512], BF16, tag=f"pt{hp}_{mk}")
                    pts.append(pt)
                    nc.scalar.activation(
                        pt[:, c0:512], tt[:, 0:w], AF.Exp, scale=SOFTCAP)
                    if o >= 0:
                        nc.vector.tensor_mul(
                            pt[:, c0:512], pt[:, c0:512], maskt[o][:, 0:w])
                    if mk == 0:
                        nc.gpsimd.tensor_copy(dn[:], pt[:])
                    else:
                        nc.gpsimd.tensor_add(
                            dn[:, c0:512], dn[:, c0:512], pt[:, c0:512])
                rb = rbp.tile([128, 512], F32, tag=f"rb{hp}")
                nc.gpsimd.partition_all_reduce(
                    rb[:], dn[:], 128, bass_isa.ReduceOp.add)
                nc.vector.reciprocal_approx_fast(rb[:], rb[:])
                return rb, pts

            def pv(nq, h, rb, pts):
                """attn[d, q] = sum_mk V[mk]^T @ P^T[mk]; drain * recip."""
                g = h // 2
                nkt = 4 * nq + 4
                ps = pv_psum.tile([128, 512], F32, tag="pv")
                for mk in range(nkt):
                    o = mk - 4 * nq
                    c0 = max(0, o) * 128
                    pt = pts[mk]
                    nc.tensor.matmul(
                        ps[:, c0:512],
                        VE[:, mk * 256 + g * 128:mk * 256 + g * 128 + 128],
                        pt[:, c0:512],
                        start=(mk == 0), stop=(mk == nkt - 1))
                at = atp.tile([128, 512], BF16, tag=f"at{h}")
                nc.vector.tensor_mul(at[:], ps[:], rb[:])
                return at

            ATT = [[None] * QH for _ in range(NQ)]
            RB = [[None] * QH for _ in range(NQ)]

            def S(nq, h):
                RB[nq][h] = scores(nq, h)

            def V(nq, h):
                rb, pts = RB[nq][h]
                ATT[nq][h] = pv(nq, h, rb, pts)
                if DEBUG_TAPS and nq == 0:
                    nc.sync.dma_start(at_dbg[h], ATT[nq][h][:])
                    nc.sync.dma_start(rb_dbg[h], rb[:])
                    if h == 0:
                        for mk in range(4):
                            c0 = mk * 128
                            nc.sync.dma_start(
                                pt_dbg[mk][:, c0:512], pts[mk][:, c0:512])

            def dma_wo(j):
                w = wop.tile([128, QH * 512], BF16, tag="wo", name="woj")
                nc.sync.dma_start(w[:], wot_d[j])
                return w

            def oproj(nq, j0, j1):
                """o_proj chunk nq for wo column-chunks j0..j1-1.

                wo tiles prefetched one j ahead so loads sit in front of the
                out-store DMAs in the SP queue.
                """
                wo_cur = dma_wo(j0)
                for j in range(j0, j1):
                    woj = wo_cur
                    if j + 1 < j1:
                        wo_cur = dma_wo(j + 1)
                    ob = obp.tile([128, 4 * 512], BF16, tag="ob", name="ob4")
                    for s in range(4):
                        po = op_psum.tile([128, 512], F32, tag="op")
                        for h in range(QH):
                            nc.tensor.matmul(
                                po[:], ATT[nq][h][:, s * 128:(s + 1) * 128],
                                woj[:, h * 512:(h + 1) * 512],
                                start=(h == 0), stop=(h == QH - 1))
                        nc.vector.tensor_copy(ob[:, s * 512:(s + 1) * 512], po[:])
                    nc.sync.dma_start(out_d[nq, j], ob[:])

            # ---- software-pipelined schedule ----
            # Each slot pairs scalar-heavy score work with PE-heavy projection
            # or o_proj streams so tanh/exp always hides under matmuls.
            proj_a(0); proj_b(0)
            if DEBUG_TAPS:
                for h in range(QH):
                    nc.sync.dma_start(qt_dbg[h], QTS[0][h][:])
            S(0, 0); S(0, 1)
            proj_a(1)
            V(0, 0); S(0, 2)
            proj_b(1)
            V(0, 1); S(0, 3)
            proj_a(2)
            V(0, 2); S(1, 0)
            proj_b(2)
            V(0, 3); S(1, 1)
            oproj(0, 0, 5)
            V(1, 0); S(1, 2)
            oproj(0, 5, 9)
            V(1, 1); S(1, 3)
            proj_a(3)
            V(1, 2); S(2, 0)
            proj_b(3)
            V(1, 3); S(2, 1)
            oproj(1, 0, 5)
            V(2, 0); S(2, 2)
            oproj(1, 5, 9)
            V(2, 1); S(2, 3)
            V(2, 2); S(3, 0)
            V(2, 3); S(3, 1)
            oproj(2, 0, 5)
            V(3, 0); S(3, 2)
            oproj(2, 5, 9)
            V(3, 1); S(3, 3)
            V(3, 2)
            V(3, 3)
            oproj(3, 0, 9)
            if DEBUG_TAPS:
                for g in range(KVH):
                    nc.sync.dma_start(kt_dbg[g], KT[g][:])
                nc.sync.dma_start(ve_dbg[:], VE[:])
    return nc


_CACHED_NC = {}


def build():
    if 0 not in _CACHED_NC:
        nc = bacc.Bacc("TRN2", target_bir_lowering=False, debug=False)
        _emit(nc)
        nc.compile()
        _CACHED_NC[0] = nc
    return _CACHED_NC[0]


def host_tables():
    inv_freq = 1.0 / (ROPE_THETA ** (np.arange(0, HEAD_DIM, 2, dtype=np.float32) / HEAD_DIM))
    ang = np.arange(L, dtype=np.float32)[:, None] * inv_freq[None, :]  # [L, 64]
    cos, sin = np.cos(ang), np.sin(ang)
    cosT = np.concatenate([cos.T, cos.T], axis=0).astype(BF16_NP)  # [128, L]
    sinT = np.concatenate([-sin.T, sin.T], axis=0).astype(BF16_NP)
    # packed [8, 128, 512]: per 256-col sub-chunk, cos cols then sin cols
    cs = np.empty((2 * NQ, 128, 512), BF16_NP)
    for s2 in range(2 * NQ):
        cs[s2, :, 0:256] = cosT[:, s2 * 256:(s2 + 1) * 256]
        cs[s2, :, 256:512] = sinT[:, s2 * 256:(s2 + 1) * 256]
    return np.ascontiguousarray(cs)


def host_masks():
    k = np.arange(128)[:, None]
    m = np.empty((128, 1280), BF16_NP)
    moff = [0, 512, 896, 1152]
    for o in range(4):
        q = np.arange(o * 128, 512)[None, :]
        m[:, moff[o]:moff[o] + 512 - o * 128] = (q >= k + 128 * o)
    return np.ascontiguousarray(m)


def _pack_kblocks(wT, width):
    """[KC*128, width] -> SBUF image [128, KC*width] (k-blocks along free)."""
    return np.ascontiguousarray(
        wT.reshape(KC, 128, width).transpose(1, 0, 2).reshape(128, KC * width))


def make_in_maps(x, wq, wk, wv, wo):
    cs = host_tables()
    masks = host_masks()
    xT = x.reshape(L, D).T.astype(BF16_NP)          # [D, L]
    # packed x: [8, 128, KC*256]: xb[s2, p, k*256+c] = xT[k*128+p, s2*256+c]
    xb = np.ascontiguousarray(
        xT.reshape(KC, 128, 2 * NQ, 256).transpose(2, 1, 0, 3)
        .reshape(2 * NQ, 128, KC * 256))
    in_maps = []
    for c in range(N_CORES):
        qs = slice(c * QH * 128, (c + 1) * QH * 128)
        kvs = slice(c * KVH * 128, (c + 1) * KVH * 128)
        woT = wo[:, qs].T.astype(BF16_NP)           # [512, D]
        # wo packed [9, 128, QH*512]: [j, p, h*512+c] = woT[h*128+p, j*512+c]
        wob = np.ascontiguousarray(
            woT.reshape(QH, 128, 9, 512).transpose(2, 1, 0, 3)
            .reshape(9, 128, QH * 512))
        in_maps.append({
            "xt": xb,
            "wqt": _pack_kblocks(wq[qs].T.astype(BF16_NP), QH * 128),
            "wkt": _pack_kblocks(wk[kvs].T.astype(BF16_NP), KVH * 128),
            "wvt": _pack_kblocks(wv[kvs].T.astype(BF16_NP), KVH * 128),
            "wot": wob,
            "cs": cs,
            "masks": masks,
        })
    return in_maps


def run(inputs, trace=False, trace_kwargs=None):
    from concourse.bass_utils import run_bass_kernel_spmd

    nc = build()
    x = np.asarray(inputs["x"], dtype=np.float32)
    in_maps = make_in_maps(
        x,
        np.asarray(inputs["wq"], dtype=np.float32),
        np.asarray(inputs["wk"], dtype=np.float32),
        np.asarray(inputs["wv"], dtype=np.float32),
        np.asarray(inputs["wo"], dtype=np.float32),
    )
    res = run_bass_kernel_spmd(
        nc, in_maps, core_ids=list(range(N_CORES)),
        trace=trace, **(trace_kwargs or {}))
    out = np.zeros((L, D), dtype=np.float32)
    for c in range(N_CORES):
        ob = res.results[c]["out"]                  # [NQ, 9, 128, 4*512]
        ob = ob.reshape(NQ, 9, 128, 4, 512).transpose(0, 3, 2, 1, 4)
        out += ob.reshape(L, D).astype(np.float32)
    return out.reshape(x.shape), res


def kernel(**inputs) -> np.ndarray:
    out, _ = run(inputs, trace=False)
    return out


# revision 33
# speedup vs baseline: 1.0556x; 1.0556x over previous
"""Trainium2 Bass kernel for GQA attention (32 q heads / 16 kv heads, head_dim
128, L=2048, D=4608) with RoPE, tanh softcap 50, causal mask, o_proj.

Strategy: tensor-parallel over heads across 8 NeuronCores. Core c computes
q-heads 4c..4c+3 and kv-heads 2c..2c+1 end-to-end; the host sums the 8 partial
[L, D] outputs (bf16 partials, f32 host accumulation).

v2 design (vs the two-phase baseline):
  - single software-pipelined pass over the 4 q-chunks of 512: causality lets
    attention for chunk nq start right after its projections (K/V history for
    chunks <= nq is already computed), so the Scalar engine's tanh+exp stream
    (~200us) hides under the PE's projection matmuls instead of serializing a
    separate attention phase
  - PV computed in [d, q] layout (lhsT = V tile, rhs = P^T tile, 512-wide
    streams) so every PE matmul streams >= 256 columns and LDWEIGHTS stays
    shadow-loaded; this also eliminates the per-128-column PE transposes of
    the attention output (o_proj consumes [d, q] directly)
  - softmax denominator accumulated on the otherwise-idle GpSimd engine
    (tensor_add over P^T tiles + partition_all_reduce broadcast), reciprocal
    on DVE, folded into the PV psum drain multiply
  - rope drains moved off the Scalar engine: DVE multiplies read the
    projection psum directly (cos/sin mul + rotate-half add)
  - wq/wk/wv resident; wo streamed per (chunk, j) to fit SBUF; x staged per
    chunk; outputs written bf16
"""

import numpy as np
import ml_dtypes

import concourse.bass as bass
import concourse.mybir as mybir
import concourse.tile as tile
from concourse import bacc, bass_isa

F32 = mybir.dt.float32
BF16 = mybir.dt.bfloat16
BF16_NP = ml_dtypes.bfloat16
AF = mybir.ActivationFunctionType

N_HEADS = 32
N_KV = 16
HEAD_DIM = 128
ROPE_THETA = 10000.0
SOFTCAP = 50.0
SCALE = 1.0 / 12.0  # 1/sqrt(144)
L = 2048
D = 4608
N_CORES = 8
QH = N_HEADS // N_CORES        # 4 local q heads
KVH = N_KV // N_CORES          # 2 local kv heads
KC = D // 128                  # 36 contraction chunks
NQ = L // 512                  # 4 l-chunks of 512
LT = L // 128                  # 16 l-tiles of 128


DEBUG_TAPS = False


def _emit(nc):
    xt_d = nc.dram_tensor("xt", [2 * NQ, 128, KC * 256], BF16, kind="ExternalInput")
    wqt_d = nc.dram_tensor("wqt", [128, KC * QH * 128], BF16, kind="ExternalInput")
    wkt_d = nc.dram_tensor("wkt", [128, KC * KVH * 128], BF16, kind="ExternalInput")
    wvt_d = nc.dram_tensor("wvt", [128, KC * KVH * 128], BF16, kind="ExternalInput")
    wot_d = nc.dram_tensor("wot", [9, 128, QH * 512], BF16, kind="ExternalInput")
    cs_d = nc.dram_tensor("cs", [2 * NQ, 128, 512], BF16, kind="ExternalInput")
    out_d = nc.dram_tensor("out", [NQ, 9, 128, 4 * 512], BF16, kind="ExternalOutput")
    if DEBUG_TAPS:
        qt_dbg = nc.dram_tensor("qt_dbg", [QH, 128, 512], BF16, kind="ExternalOutput")
        kt_dbg = nc.dram_tensor("kt_dbg", [KVH, 128, L], BF16, kind="ExternalOutput")
        ve_dbg = nc.dram_tensor("ve_dbg", [128, LT * 256], BF16, kind="ExternalOutput")
        at_dbg = nc.dram_tensor("at_dbg", [QH, 128, 512], BF16, kind="ExternalOutput")
        rb_dbg = nc.dram_tensor("rb_dbg", [QH, 128, 512], F32, kind="ExternalOutput")
        dn_dbg = nc.dram_tensor("dn_dbg", [128, 512], F32, kind="ExternalOutput")
        pr_dbg = nc.dram_tensor("pr_dbg", [128, 512], F32, kind="ExternalOutput")
        pt_dbg = nc.dram_tensor("pt_dbg", [4, 128, 512], BF16, kind="ExternalOutput")

    with tile.TileContext(nc) as tc:
        with (
            tc.tile_pool(name="const", bufs=1) as const,
            tc.tile_pool(name="wts", bufs=1) as wts,
            tc.tile_pool(name="wo", bufs=2) as wop,
            tc.tile_pool(name="xp", bufs=2) as xp,
            tc.tile_pool(name="cs", bufs=1) as csp,
            tc.tile_pool(name="qt", bufs=2) as qtp,
            tc.tile_pool(name="persist", bufs=1) as persist,
            tc.tile_pool(name="pt", bufs=1) as ptp,
            tc.tile_pool(name="rp", bufs=1) as rpp,
            tc.tile_pool(name="tt", bufs=1) as ttp,
            tc.tile_pool(name="dn", bufs=1) as dnp,
            tc.tile_pool(name="pr", bufs=1) as prp,
            tc.tile_pool(name="rb", bufs=2) as rbp,
            tc.tile_pool(name="at", bufs=3) as atp,
            tc.tile_pool(name="ob", bufs=1) as obp,
            tc.tile_pool(name="pj_psum", bufs=2, space="PSUM") as pj_psum,
            tc.tile_pool(name="sc_psum", bufs=2, space="PSUM") as sc_psum,
            tc.tile_pool(name="pv_psum", bufs=2, space="PSUM") as pv_psum,
            tc.tile_pool(name="op_psum", bufs=2, space="PSUM") as op_psum,
        ):
            # ---- persistent tensors ----
            KT = [persist.tile([128, L], BF16, tag=f"kt{g}", name=f"kt{g}")
                  for g in range(KVH)]
            VE = persist.tile([128, LT * 256], BF16, tag="ve", name="ve")
            QTS = [[None] * QH for _ in range(NQ)]
            # ---- prologue DMA: one 2D descriptor per tensor (descriptor
            # service pace is ~2us each regardless of size; count rules) ----
            def dma_x(s2):
                t = xp.tile([128, KC * 256], BF16, tag="x", name="xc")
                nc.sync.dma_start(t[:], xt_d[s2])
                return t

            xc0 = dma_x(0)
            wk = wts.tile([128, KC * KVH * 128], BF16, tag="wk", name="wk")
            nc.sync.dma_start(wk[:], wkt_d[:])
            wq = wts.tile([128, KC * QH * 128], BF16, tag="wq", name="wq")
            nc.sync.dma_start(wq[:], wqt_d[:])
            wv = wts.tile([128, KC * KVH * 128], BF16, tag="wv", name="wv")
            nc.sync.dma_start(wv[:], wvt_d[:])

            def dma_cs(s2):
                t = csp.tile([128, 512], BF16, tag="cs", name="cs")
                nc.sync.dma_start(t[:], cs_d[s2])
                return t[:, 0:256], t[:, 256:512]

            x_next = [xc0]

            def rope_drain(ps, dst, cosc, sinc):
                """psum [128,256] f32 -> rotate-half rope -> dst bf16."""
                t1 = rpp.tile([128, 256], F32, tag="r1")
                nc.vector.tensor_mul(t1[:], ps[:], cosc[:])
                t2 = rpp.tile([128, 256], F32, tag="r2")
                nc.vector.tensor_mul(t2[0:64, :], ps[64:128, :], sinc[0:64, :])
                nc.vector.tensor_mul(t2[64:128, :], ps[0:64, :], sinc[64:128, :])
                nc.vector.tensor_add(dst[:], t1[:], t2[:])

            def proj_sub(s2):
                """Projections for 256-col sub-chunk s2 (K, Q, V + rope).

                Prefetches sub-chunk s2+1's x tiles (bufs=2 ring, no WAR
                wait) so projection matmuls never stall on staging DMA.
                """
                nq, half = s2 // 2, s2 % 2
                xc = x_next[0]
                if s2 + 1 < 2 * NQ:
                    x_next[0] = dma_x(s2 + 1)
                cosc, sinc = dma_cs(s2)
                cols = slice(half * 256, half * 256 + 256)
                for g in range(KVH):
                    ps = pj_psum.tile([128, 256], F32, tag="pj")
                    for k in range(KC):
                        nc.tensor.matmul(
                            ps[:], wk[:, k * 256 + g * 128:k * 256 + g * 128 + 128],
                            xc[:, k * 256:(k + 1) * 256],
                            start=(k == 0), stop=(k == KC - 1))
                    rope_drain(ps, KT[g][:, s2 * 256:(s2 + 1) * 256],
                               cosc, sinc)
                for h in range(QH):
                    if half == 0:
                        QTS[nq][h] = qtp.tile([128, 512], BF16, tag=f"qt{h}", name=f"qt{h}")
                    qt = QTS[nq][h]
                    ps = pj_psum.tile([128, 256], F32, tag="pj")
                    for k in range(KC):
                        nc.tensor.matmul(
                            ps[:], wq[:, k * 512 + h * 128:k * 512 + h * 128 + 128],
                            xc[:, k * 256:(k + 1) * 256],
                            start=(k == 0), stop=(k == KC - 1))
                    rope_drain(ps, qt[:, cols], cosc, sinc)
                for b in range(2):
                    mk = s2 * 2 + b
                    ps = pj_psum.tile([128, 256], F32, tag="pj")
                    for k in range(KC):
                        nc.tensor.matmul(
                            ps[:], xc[:, k * 256 + b * 128:k * 256 + b * 128 + 128],
                            wv[:, k * 256:(k + 1) * 256],
                            start=(k == 0), stop=(k == KC - 1))
                    nc.vector.tensor_copy(
                        VE[:, mk * 256:(mk + 1) * 256], ps[:])

            def proj_a(nq):
                proj_sub(2 * nq)

            def proj_b(nq):
                proj_sub(2 * nq + 1)

            def scores(nq, h):
                """scores -> tanh -> exp -> mask; GpSimd denom; rb recip."""
                g = h // 2
                nkt = 4 * nq + 4
                hp = h % 2
                pts = []
                dn = dnp.tile([128, 512], F32, tag="dn")
                for mk in range(nkt):
                    o = mk - 4 * nq
                    c0 = max(0, o) * 128
                    w = 512 - c0
                    ps_s = sc_psum.tile([128, 512], F32, tag="sc")
                    nc.tensor.matmul(
                        ps_s[:, 0:w], KT[g][:, mk * 128:(mk + 1) * 128],
                        QTS[nq][h][:, c0:512])
                    tt = ttp.tile([128, 512], F32, tag="tanh")
                    nc.scalar.activation(
                        tt[:, 0:w], ps_s[:, 0:w], AF.Tanh, scale=SCALE / SOFTCAP)
                    pt = ptp.tile([128, 512], BF16, tag=f"pt{hp}_{mk}")
                    pts.append(pt)
                    nc.scalar.activation(
                        pt[:, c0:512], tt[:, 0:w], AF.Exp, scale=SOFTCAP)
                    if o >= 0:
                        # causal: keep col j (q = c0+j) iff j - k >= 0
                        nc.gpsimd.affine_select(
                            pt[:, c0:512], pt[:, c0:512],
                            pattern=[[1, w]],
                            compare_op=mybir.AluOpType.is_ge,
                            fill=0.0, base=0, channel_multiplier=-1)
                    if DEBUG_TAPS and nq == 0 and h == 0:
                        nc.sync.dma_start(
                            pt_dbg[mk][:, c0:512], pt[:, c0:512])
                    if mk == 0:
                        nc.gpsimd.tensor_copy(dn[:], pt[:])
                    else:
                        nc.gpsimd.tensor_add(
                            dn[:, c0:512], dn[:, c0:512], pt[:, c0:512])
                if DEBUG_TAPS and nq == 0 and h == 0:
                    nc.sync.dma_start(dn_dbg[:], dn[:])
                pr = prp.tile([128, 512], F32, tag="pr")
                nc.gpsimd.partition_all_reduce(
                    pr[:], dn[:], 128, bass_isa.ReduceOp.add)
                if DEBUG_TAPS and nq == 0 and h == 0:
                    nc.sync.dma_start(pr_dbg[:], pr[:])
                rb = rbp.tile([128, 512], F32, tag=f"rb{hp}")
                nc.vector.reciprocal_approx_fast(rb[:], pr[:])
                if DEBUG_TAPS and nq == 0 and h == 0:
                    nc.sync.dma_start(rb_dbg[0], rb[:])
                return rb, pts

            def pv(nq, h, rb, pts):
                """attn[d, q] = sum_mk V[mk]^T @ P^T[mk]; drain * recip."""
                g = h // 2
                nkt = 4 * nq + 4
                ps = pv_psum.tile([128, 512], F32, tag="pv")
                for mk in range(nkt):
                    o = mk - 4 * nq
                    c0 = max(0, o) * 128
                    pt = pts[mk]
                    nc.tensor.matmul(
                        ps[:, c0:512],
                        VE[:, mk * 256 + g * 128:mk * 256 + g * 128 + 128],
                        pt[:, c0:512],
                        start=(mk == 0), stop=(mk == nkt - 1))
                at = atp.tile([128, 512], BF16, tag=f"at{h}")
                nc.vector.tensor_copy(at[:], ps[:])
                if DEBUG_TAPS and nq == 0 and h == 0:
                    nc.sync.dma_start(at_dbg[1], at[:])
                return at

            ATT = [[None] * QH for _ in range(NQ)]
            RB = [[None] * QH for _ in range(NQ)]

            def S(nq, h):
                RB[nq][h] = scores(nq, h)

            def V(nq, h):
                rb, pts = RB[nq][h]
                ATT[nq][h] = pv(nq, h, rb, pts)

            def N(nq, h):
                """Deferred normalization: attnT_norm = attnT_raw * 1/denom."""
                rb, _ = RB[nq][h]
                raw = ATT[nq][h]
                if DEBUG_TAPS and nq == 0 and h == 0:
                    nc.sync.dma_start(rb_dbg[2], rb[:])
                    nc.sync.dma_start(at_dbg[2], raw[:])
                at = atp.tile([128, 512], BF16, tag=f"at{h}", name=f"at{h}")
                nc.vector.tensor_mul(at[:], raw[:], rb[:])
                ATT[nq][h] = at
                if DEBUG_TAPS and nq == 0 and h == 0:
                    nc.sync.dma_start(at_dbg[0], at[:])
                if DEBUG_TAPS and nq == 0:
                    nc.sync.dma_start(at_dbg[h], ATT[nq][h][:])
                    nc.sync.dma_start(rb_dbg[h], rb[:])
                    pass

            def dma_wo(j):
                w = wop.tile([128, QH * 512], BF16, tag="wo", name="woj")
                nc.sync.dma_start(w[:], wot_d[j])
                return w

            def oproj(nq, j0, j1):
                """o_proj chunk nq for wo column-chunks j0..j1-1.

                wo tiles prefetched one j ahead so loads sit in front of the
                out-store DMAs in the SP queue.
                """
                wo_cur = dma_wo(j0)
                for j in range(j0, j1):
                    woj = wo_cur
                    if j + 1 < j1:
                        wo_cur = dma_wo(j + 1)
                    ob = obp.tile([128, 4 * 512], BF16, tag="ob", name="ob4")
                    for s in range(4):
                        po = op_psum.tile([128, 512], F32, tag="op")
                        for h in range(QH):
                            nc.tensor.matmul(
                                po[:], ATT[nq][h][:, s * 128:(s + 1) * 128],
                                woj[:, h * 512:(h + 1) * 512],
                                start=(h == 0), stop=(h == QH - 1))
                        nc.vector.tensor_copy(ob[:, s * 512:(s + 1) * 512], po[:])
                    nc.sync.dma_start(out_d[nq, j], ob[:])

            # ---- software-pipelined schedule ----
            # Each slot pairs scalar-heavy score work with PE-heavy projection
            # or o_proj streams so tanh/exp always hides under matmuls.
            proj_a(0); proj_b(0)
            if DEBUG_TAPS:
                for h in range(QH):
                    nc.sync.dma_start(qt_dbg[h], QTS[0][h][:])
            S(0, 0); S(0, 1)
            proj_a(1)
            V(0, 0); S(0, 2); N(0, 0)
            proj_b(1)
            V(0, 1); S(0, 3); N(0, 1)
            proj_a(2)
            V(0, 2); S(1, 0); N(0, 2)
            proj_b(2)
            V(0, 3); S(1, 1); N(0, 3)
            oproj(0, 0, 5)
            V(1, 0); S(1, 2); N(1, 0)
            oproj(0, 5, 9)
            V(1, 1); S(1, 3); N(1, 1)
            proj_a(3)
            V(1, 2); S(2, 0); N(1, 2)
            proj_b(3)
            V(1, 3); S(2, 1); N(1, 3)
            oproj(1, 0, 5)
            V(2, 0); S(2, 2); N(2, 0)
            oproj(1, 5, 9)
            V(2, 1); S(2, 3); N(2, 1)
            V(2, 2); S(3, 0); N(2, 2)
            V(2, 3); S(3, 1); N(2, 3)
            oproj(2, 0, 5)
            V(3, 0); S(3, 2); N(3, 0)
            oproj(2, 5, 9)
            V(3, 1); S(3, 3); N(3, 1)
            V(3, 2); N(3, 2)
            V(3, 3); N(3, 3)
            oproj(3, 0, 9)
            if DEBUG_TAPS:
                for g in range(KVH):
                    nc.sync.dma_start(kt_dbg[g], KT[g][:])
                nc.sync.dma_start(ve_dbg[:], VE[:])
    return nc


_CACHED_NC = {}


def build():
    if 0 not in _CACHED_NC:
        nc = bacc.Bacc("TRN2", target_bir_lowering=False, debug=False)
        _emit(nc)
        nc.compile()
        _CACHED_NC[0] = nc
    return _CACHED_NC[0]


def host_tables():
    inv_freq = 1.0 / (ROPE_THETA ** (np.arange(0, HEAD_DIM, 2, dtype=np.float32) / HEAD_DIM))
    ang = np.arange(L, dtype=np.float32)[:, None] * inv_freq[None, :]  # [L, 64]
    cos, sin = np.cos(ang), np.sin(ang)
    cosT = np.concatenate([cos.T, cos.T], axis=0).astype(BF16_NP)  # [128, L]
    sinT = np.concatenate([-sin.T, sin.T], axis=0).astype(BF16_NP)
    # packed [8, 128, 512]: per 256-col sub-chunk, cos cols then sin cols
    cs = np.empty((2 * NQ, 128, 512), BF16_NP)
    for s2 in range(2 * NQ):
        cs[s2, :, 0:256] = cosT[:, s2 * 256:(s2 + 1) * 256]
        cs[s2, :, 256:512] = sinT[:, s2 * 256:(s2 + 1) * 256]
    return np.ascontiguousarray(cs)


def _pack_kblocks(wT, width):
    """[KC*128, width] -> SBUF image [128, KC*width] (k-blocks along free)."""
    return np.ascontiguousarray(
        wT.reshape(KC, 128, width).transpose(1, 0, 2).reshape(128, KC * width))


def make_in_maps(x, wq, wk, wv, wo):
    cs = host_tables()
    xT = x.reshape(L, D).T.astype(BF16_NP)          # [D, L]
    # packed x: [8, 128, KC*256]: xb[s2, p, k*256+c] = xT[k*128+p, s2*256+c]
    xb = np.ascontiguousarray(
        xT.reshape(KC, 128, 2 * NQ, 256).transpose(2, 1, 0, 3)
        .reshape(2 * NQ, 128, KC * 256))
    in_maps = []
    for c in range(N_CORES):
        qs = slice(c * QH * 128, (c + 1) * QH * 128)
        kvs = slice(c * KVH * 128, (c + 1) * KVH * 128)
        woT = wo[:, qs].T.astype(BF16_NP)           # [512, D]
        # wo packed [9, 128, QH*512]: [j, p, h*512+c] = woT[h*128+p, j*512+c]
        wob = np.ascontiguousarray(
            woT.reshape(QH, 128, 9, 512).transpose(2, 1, 0, 3)
            .reshape(9, 128, QH * 512))
        in_maps.append({
            "xt": xb,
            "wqt": _pack_kblocks(wq[qs].T.astype(BF16_NP), QH * 128),
            "wkt": _pack_kblocks(wk[kvs].T.astype(BF16_NP), KVH * 128),
            "wvt": _pack_kblocks(wv[kvs].T.astype(BF16_NP), KVH * 128),
            "wot": wob,
            "cs": cs,
        })
    return in_maps


def run(inputs, trace=False, trace_kwargs=None):
    from concourse.bass_utils import run_bass_kernel_spmd

    nc = build()
    x = np.asarray(inputs["x"], dtype=np.float32)
    in_maps = make_in_maps(
        x,
        np.asarray(inputs["wq"], dtype=np.float32),
        np.asarray(inputs["wk"], dtype=np.float32),
        np.asarray(inputs["wv"], dtype=np.float32),
        np.asarray(inputs["wo"], dtype=np.float32),
    )
    res = run_bass_kernel_spmd(
        nc, in_maps, core_ids=list(range(N_CORES)),
        trace=trace, **(trace_kwargs or {}))
    out = np.zeros((L, D), dtype=np.float32)
    for c in range(N_CORES):
        ob = res.results[c]["out"]                  # [NQ, 9, 128, 4*512]
        ob = ob.reshape(NQ, 9, 128, 4, 512).transpose(0, 3, 2, 1, 4)
        out += ob.reshape(L, D).astype(np.float32)
    return out.reshape(x.shape), res


def kernel(**inputs) -> np.ndarray:
    out, _ = run(inputs, trace=False)
    return out


# revision 37
# speedup vs baseline: 1.3348x; 1.2645x over previous
"""Trainium2 Bass kernel for GQA attention (32 q heads / 16 kv heads, head_dim
128, L=2048, D=4608) with RoPE, tanh softcap 50, causal mask, o_proj.

Strategy: tensor-parallel over heads across 8 NeuronCores. Core c computes
q-heads 4c..4c+3 and kv-heads 2c..2c+1 end-to-end; the host sums the 8 partial
[L, D] outputs (bf16 partials, f32 host accumulation).

v2 design (vs the two-phase baseline):
  - single software-pipelined pass over the 4 q-chunks of 512: causality lets
    attention for chunk nq start right after its projections (K/V history for
    chunks <= nq is already computed), so the Scalar engine's tanh+exp stream
    (~200us) hides under the PE's projection matmuls instead of serializing a
    separate attention phase
  - PV computed in [d, q] layout (lhsT = V tile, rhs = P^T tile, 512-wide
    streams) so every PE matmul streams >= 256 columns and LDWEIGHTS stays
    shadow-loaded; this also eliminates the per-128-column PE transposes of
    the attention output (o_proj consumes [d, q] directly)
  - softmax denominator accumulated on the otherwise-idle GpSimd engine
    (tensor_add over P^T tiles + partition_all_reduce broadcast), reciprocal
    on DVE, folded into the PV psum drain multiply
  - rope drains moved off the Scalar engine: DVE multiplies read the
    projection psum directly (cos/sin mul + rotate-half add)
  - wq/wk/wv resident; wo streamed per (chunk, j) to fit SBUF; x staged per
    chunk; outputs written bf16
"""

import numpy as np
import ml_dtypes

import concourse.bass as bass
import concourse.mybir as mybir
import concourse.tile as tile
from concourse import bacc, bass_isa

F32 = mybir.dt.float32
BF16 = mybir.dt.bfloat16
BF16_NP = ml_dtypes.bfloat16
AF = mybir.ActivationFunctionType

N_HEADS = 32
N_KV = 16
HEAD_DIM = 128
ROPE_THETA = 10000.0
SOFTCAP = 50.0
SCALE = 1.0 / 12.0  # 1/sqrt(144)
L = 2048
D = 4608
N_CORES = 8
QH = N_HEADS // N_CORES        # 4 local q heads
KVH = N_KV // N_CORES          # 2 local kv heads
KC = D // 128                  # 36 contraction chunks
NQ = L // 512                  # 4 l-chunks of 512
LT = L // 128                  # 16 l-tiles of 128


DEBUG_TAPS = False


def _emit(nc):
    xt_d = nc.dram_tensor("xt", [2 * NQ, 128, KC * 256], BF16, kind="ExternalInput")
    wqt_d = nc.dram_tensor("wqt", [128, KC * QH * 128], BF16, kind="ExternalInput")
    wkt_d = nc.dram_tensor("wkt", [128, KC * KVH * 128], BF16, kind="ExternalInput")
    wvt_d = nc.dram_tensor("wvt", [128, KC * KVH * 128], BF16, kind="ExternalInput")
    wot_d = nc.dram_tensor("wot", [9, 128, QH * 512], BF16, kind="ExternalInput")
    cs_d = nc.dram_tensor("cs", [2 * NQ, 128, 512], BF16, kind="ExternalInput")
    out_d = nc.dram_tensor("out", [NQ, 9, 128, 4 * 512], BF16, kind="ExternalOutput")
    if DEBUG_TAPS:
        qt_dbg = nc.dram_tensor("qt_dbg", [QH, 128, 512], BF16, kind="ExternalOutput")
        kt_dbg = nc.dram_tensor("kt_dbg", [KVH, 128, L], BF16, kind="ExternalOutput")
        ve_dbg = nc.dram_tensor("ve_dbg", [128, LT * 256], BF16, kind="ExternalOutput")
        at_dbg = nc.dram_tensor("at_dbg", [QH, 128, 512], BF16, kind="ExternalOutput")
        rb_dbg = nc.dram_tensor("rb_dbg", [QH, 128, 512], F32, kind="ExternalOutput")
        dn_dbg = nc.dram_tensor("dn_dbg", [128, 512], F32, kind="ExternalOutput")
        pr_dbg = nc.dram_tensor("pr_dbg", [128, 512], F32, kind="ExternalOutput")
        pt_dbg = nc.dram_tensor("pt_dbg", [4, 128, 512], BF16, kind="ExternalOutput")

    with tile.TileContext(nc) as tc:
        with (
            tc.tile_pool(name="const", bufs=1) as const,
            tc.tile_pool(name="wts", bufs=1) as wts,
            tc.tile_pool(name="wo", bufs=2) as wop,
            tc.tile_pool(name="xp", bufs=2) as xp,
            tc.tile_pool(name="cs", bufs=1) as csp,
            tc.tile_pool(name="qt", bufs=2) as qtp,
            tc.tile_pool(name="persist", bufs=1) as persist,
            tc.tile_pool(name="pt", bufs=1) as ptp,
            tc.tile_pool(name="rp", bufs=1) as rpp,
            tc.tile_pool(name="tt", bufs=1) as ttp,
            tc.tile_pool(name="rb", bufs=2) as rbp,
            tc.tile_pool(name="at", bufs=3) as atp,
            tc.tile_pool(name="ob", bufs=1) as obp,
            tc.tile_pool(name="pj_psum", bufs=2, space="PSUM") as pj_psum,
            tc.tile_pool(name="sc_psum", bufs=2, space="PSUM") as sc_psum,
            tc.tile_pool(name="pv_psum", bufs=1, space="PSUM") as pv_psum,
            tc.tile_pool(name="op_psum", bufs=2, space="PSUM") as op_psum,
        ):
            # ---- persistent tensors ----
            KT = [persist.tile([128, L], BF16, tag=f"kt{g}", name=f"kt{g}")
                  for g in range(KVH)]
            VE = persist.tile([128, LT * 256], BF16, tag="ve", name="ve")
            QTS = [[None] * QH for _ in range(NQ)]
            # ---- prologue DMA: one 2D descriptor per tensor (descriptor
            # service pace is ~2us each regardless of size; count rules) ----
            def dma_x(s2):
                t = xp.tile([128, KC * 256], BF16, tag="x", name="xc")
                nc.sync.dma_start(t[:], xt_d[s2])
                return t

            ones = const.tile([128, 128], BF16, tag="ones")
            nc.vector.memset(ones[:], 1.0)
            xc0 = dma_x(0)
            wk = wts.tile([128, KC * KVH * 128], BF16, tag="wk", name="wk")
            nc.sync.dma_start(wk[:], wkt_d[:])
            wq = wts.tile([128, KC * QH * 128], BF16, tag="wq", name="wq")
            nc.sync.dma_start(wq[:], wqt_d[:])
            wv = wts.tile([128, KC * KVH * 128], BF16, tag="wv", name="wv")
            nc.sync.dma_start(wv[:], wvt_d[:])

            def dma_cs(s2):
                t = csp.tile([128, 512], BF16, tag="cs", name="cs")
                nc.sync.dma_start(t[:], cs_d[s2])
                return t[:, 0:256], t[:, 256:512]

            x_next = [xc0]

            def rope_drain(ps, dst, cosc, sinc):
                """psum [128,256] f32 -> rotate-half rope -> dst bf16."""
                t1 = rpp.tile([128, 256], F32, tag="r1")
                nc.vector.tensor_mul(t1[:], ps[:], cosc[:])
                t2 = rpp.tile([128, 256], F32, tag="r2")
                nc.vector.tensor_mul(t2[0:64, :], ps[64:128, :], sinc[0:64, :])
                nc.vector.tensor_mul(t2[64:128, :], ps[0:64, :], sinc[64:128, :])
                nc.vector.tensor_add(dst[:], t1[:], t2[:])

            def proj_sub(s2):
                """Projections for 256-col sub-chunk s2 (K, Q, V + rope).

                Prefetches sub-chunk s2+1's x tiles (bufs=2 ring, no WAR
                wait) so projection matmuls never stall on staging DMA.
                """
                nq, half = s2 // 2, s2 % 2
                xc = x_next[0]
                if s2 + 1 < 2 * NQ:
                    x_next[0] = dma_x(s2 + 1)
                cosc, sinc = dma_cs(s2)
                cols = slice(half * 256, half * 256 + 256)
                for g in range(KVH):
                    ps = pj_psum.tile([128, 256], F32, tag="pj")
                    for k in range(KC):
                        nc.tensor.matmul(
                            ps[:], wk[:, k * 256 + g * 128:k * 256 + g * 128 + 128],
                            xc[:, k * 256:(k + 1) * 256],
                            start=(k == 0), stop=(k == KC - 1))
                    rope_drain(ps, KT[g][:, s2 * 256:(s2 + 1) * 256],
                               cosc, sinc)
                for h in range(QH):
                    if half == 0:
                        QTS[nq][h] = qtp.tile([128, 512], BF16, tag=f"qt{h}", name=f"qt{h}")
                    qt = QTS[nq][h]
                    ps = pj_psum.tile([128, 256], F32, tag="pj")
                    for k in range(KC):
                        nc.tensor.matmul(
                            ps[:], wq[:, k * 512 + h * 128:k * 512 + h * 128 + 128],
                            xc[:, k * 256:(k + 1) * 256],
                            start=(k == 0), stop=(k == KC - 1))
                    rope_drain(ps, qt[:, cols], cosc, sinc)
                for b in range(2):
                    mk = s2 * 2 + b
                    ps = pj_psum.tile([128, 256], F32, tag="pj")
                    for k in range(KC):
                        nc.tensor.matmul(
                            ps[:], xc[:, k * 256 + b * 128:k * 256 + b * 128 + 128],
                            wv[:, k * 256:(k + 1) * 256],
                            start=(k == 0), stop=(k == KC - 1))
                    nc.vector.tensor_copy(
                        VE[:, mk * 256:(mk + 1) * 256], ps[:])

            def proj_a(nq):
                proj_sub(2 * nq)

            def proj_b(nq):
                proj_sub(2 * nq + 1)

            def scores(nq, h):
                """scores -> tanh -> exp -> causal mask (affine_select)."""
                g = h // 2
                nkt = 4 * nq + 4
                hp = h % 2
                pts = []
                for mk in range(nkt):
                    o = mk - 4 * nq
                    c0 = max(0, o) * 128
                    w = 512 - c0
                    ps_s = sc_psum.tile([128, 512], F32, tag="sc")
                    nc.tensor.matmul(
                        ps_s[:, 0:w], KT[g][:, mk * 128:(mk + 1) * 128],
                        QTS[nq][h][:, c0:512])
                    tt = ttp.tile([128, 512], F32, tag="tanh")
                    nc.scalar.activation(
                        tt[:, 0:w], ps_s[:, 0:w], AF.Tanh, scale=SCALE / SOFTCAP)
                    pt = ptp.tile([128, 512], BF16, tag=f"pt{hp}_{mk}")
                    pts.append(pt)
                    nc.scalar.activation(
                        pt[:, c0:512], tt[:, 0:w], AF.Exp, scale=SOFTCAP)
                    if o >= 0:
                        # causal: keep col j (q = c0+j) iff j - k >= 0
                        nc.gpsimd.affine_select(
                            pt[:, c0:512], pt[:, c0:512],
                            pattern=[[1, w]],
                            compare_op=mybir.AluOpType.is_ge,
                            fill=0.0, base=0, channel_multiplier=-1)
                return pts

            def pv(nq, h, pts):
                """attn[d, q] = sum_mk V[mk]^T @ P^T[mk]; denom = ones^T P."""
                g = h // 2
                nkt = 4 * nq + 4
                hp = h % 2
                ps = pv_psum.tile([128, 512], F32, tag="pv")
                for mk in range(nkt):
                    o = mk - 4 * nq
                    c0 = max(0, o) * 128
                    nc.tensor.matmul(
                        ps[:, c0:512],
                        VE[:, mk * 256 + g * 128:mk * 256 + g * 128 + 128],
                        pts[mk][:, c0:512],
                        start=(mk == 0), stop=(mk == nkt - 1))
                at = atp.tile([128, 512], BF16, tag=f"at{h}")
                nc.vector.tensor_copy(at[:], ps[:])
                dn = pv_psum.tile([128, 512], F32, tag="dn")
                for mk in range(nkt):
                    o = mk - 4 * nq
                    c0 = max(0, o) * 128
                    nc.tensor.matmul(
                        dn[:, c0:512], ones[:], pts[mk][:, c0:512],
                        start=(mk == 0), stop=(mk == nkt - 1))
                rb = rbp.tile([128, 512], F32, tag=f"rb{hp}")
                nc.vector.reciprocal_approx_fast(rb[:], dn[:])
                return at, rb

            ATT = [[None] * QH for _ in range(NQ)]
            RB = [[None] * QH for _ in range(NQ)]

            def S(nq, h):
                RB[nq][h] = scores(nq, h)

            def V(nq, h):
                pts = RB[nq][h]
                ATT[nq][h] = pv(nq, h, pts)

            def N(nq, h):
                """Deferred normalization: attnT_norm = attnT_raw * 1/denom."""
                raw, rb = ATT[nq][h]
                at = atp.tile([128, 512], BF16, tag=f"at{h}", name=f"at{h}")
                nc.vector.tensor_mul(at[:], raw[:], rb[:])
                ATT[nq][h] = at

            def dma_wo(j):
                w = wop.tile([128, QH * 512], BF16, tag="wo", name="woj")
                nc.sync.dma_start(w[:], wot_d[j])
                return w

            def oproj(nq, j0, j1):
                """o_proj chunk nq for wo column-chunks j0..j1-1.

                wo tiles prefetched one j ahead so loads sit in front of the
                out-store DMAs in the SP queue.
                """
                wo_cur = dma_wo(j0)
                for j in range(j0, j1):
                    woj = wo_cur
                    if j + 1 < j1:
                        wo_cur = dma_wo(j + 1)
                    ob = obp.tile([128, 4 * 512], BF16, tag="ob", name="ob4")
                    for s in range(4):
                        po = op_psum.tile([128, 512], F32, tag="op")
                        for h in range(QH):
                            nc.tensor.matmul(
                                po[:], ATT[nq][h][:, s * 128:(s + 1) * 128],
                                woj[:, h * 512:(h + 1) * 512],
                                start=(h == 0), stop=(h == QH - 1))
                        nc.vector.tensor_copy(ob[:, s * 512:(s + 1) * 512], po[:])
                    nc.sync.dma_start(out_d[nq, j], ob[:])

            # ---- software-pipelined schedule ----
            # Each slot pairs scalar-heavy score work with PE-heavy projection
            # or o_proj streams so tanh/exp always hides under matmuls.
            proj_a(0); proj_b(0)
            if DEBUG_TAPS:
                for h in range(QH):
                    nc.sync.dma_start(qt_dbg[h], QTS[0][h][:])
            S(0, 0); S(0, 1)
            proj_a(1)
            V(0, 0); S(0, 2); N(0, 0)
            proj_b(1)
            V(0, 1); S(0, 3); N(0, 1)
            proj_a(2)
            V(0, 2); S(1, 0); N(0, 2)
            proj_b(2)
            V(0, 3); S(1, 1); N(0, 3)
            oproj(0, 0, 5)
            V(1, 0); S(1, 2); N(1, 0)
            oproj(0, 5, 9)
            V(1, 1); S(1, 3); N(1, 1)
            proj_a(3)
            V(1, 2); S(2, 0); N(1, 2)
            proj_b(3)
            V(1, 3); S(2, 1); N(1, 3)
            oproj(1, 0, 5)
            V(2, 0); S(2, 2); N(2, 0)
            oproj(1, 5, 9)
            V(2, 1); S(2, 3); N(2, 1)
            V(2, 2); S(3, 0); N(2, 2)
            V(2, 3); S(3, 1); N(2, 3)
            oproj(2, 0, 5)
            V(3, 0); S(3, 2); N(3, 0)
            oproj(2, 5, 9)
            V(3, 1); S(3, 3); N(3, 1)
            V(3, 2); N(3, 2)
            V(3, 3); N(3, 3)
            oproj(3, 0, 9)
            if DEBUG_TAPS:
                for g in range(KVH):
                    nc.sync.dma_start(kt_dbg[g], KT[g][:])
                nc.sync.dma_start(ve_dbg[:], VE[:])
    return nc


_CACHED_NC = {}


def build():
    if 0 not in _CACHED_NC:
        nc = bacc.Bacc("TRN2", target_bir_lowering=False, debug=False)
        _emit(nc)
        nc.compile()
        _CACHED_NC[0] = nc
    return _CACHED_NC[0]


def host_tables():
    inv_freq = 1.0 / (ROPE_THETA ** (np.arange(0, HEAD_DIM, 2, dtype=np.float32) / HEAD_DIM))
    ang = np.arange(L, dtype=np.float32)[:, None] * inv_freq[None, :]  # [L, 64]
    cos, sin = np.cos(ang), np.sin(ang)
    cosT = np.concatenate([cos.T, cos.T], axis=0).astype(BF16_NP)  # [128, L]
    sinT = np.concatenate([-sin.T, sin.T], axis=0).astype(BF16_NP)
    # packed [8, 128, 512]: per 256-col sub-chunk, cos cols then sin cols
    cs = np.empty((2 * NQ, 128, 512), BF16_NP)
    for s2 in range(2 * NQ):
        cs[s2, :, 0:256] = cosT[:, s2 * 256:(s2 + 1) * 256]
        cs[s2, :, 256:512] = sinT[:, s2 * 256:(s2 + 1) * 256]
    return np.ascontiguousarray(cs)


def _pack_kblocks(wT, width):
    """[KC*128, width] -> SBUF image [128, KC*width] (k-blocks along free)."""
    return np.ascontiguousarray(
        wT.reshape(KC, 128, width).transpose(1, 0, 2).reshape(128, KC * width))


def make_in_maps(x, wq, wk, wv, wo):
    cs = host_tables()
    xT = x.reshape(L, D).T.astype(BF16_NP)          # [D, L]
    # packed x: [8, 128, KC*256]: xb[s2, p, k*256+c] = xT[k*128+p, s2*256+c]
    xb = np.ascontiguousarray(
        xT.reshape(KC, 128, 2 * NQ, 256).transpose(2, 1, 0, 3)
        .reshape(2 * NQ, 128, KC * 256))
    in_maps = []
    for c in range(N_CORES):
        qs = slice(c * QH * 128, (c + 1) * QH * 128)
        kvs = slice(c * KVH * 128, (c + 1) * KVH * 128)
        woT = wo[:, qs].T.astype(BF16_NP)           # [512, D]
        # wo packed [9, 128, QH*512]: [j, p, h*512+c] = woT[h*128+p, j*512+c]
        wob = np.ascontiguousarray(
            woT.reshape(QH, 128, 9, 512).transpose(2, 1, 0, 3)
            .reshape(9, 128, QH * 512))
        in_maps.append({
            "xt": xb,
            "wqt": _pack_kblocks(wq[qs].T.astype(BF16_NP), QH * 128),
            "wkt": _pack_kblocks(wk[kvs].T.astype(BF16_NP), KVH * 128),
            "wvt": _pack_kblocks(wv[kvs].T.astype(BF16_NP), KVH * 128),
            "wot": wob,
            "cs": cs,
        })
    return in_maps


def run(inputs, trace=False, trace_kwargs=None):
    from concourse.bass_utils import run_bass_kernel_spmd

    nc = build()
    x = np.asarray(inputs["x"], dtype=np.float32)
    in_maps = make_in_maps(
        x,
        np.asarray(inputs["wq"], dtype=np.float32),
        np.asarray(inputs["wk"], dtype=np.float32),
        np.asarray(inputs["wv"], dtype=np.float32),
        np.asarray(inputs["wo"], dtype=np.float32),
    )
    res = run_bass_kernel_spmd(
        nc, in_maps, core_ids=list(range(N_CORES)),
        trace=trace, **(trace_kwargs or {}))
    out = np.zeros((L, D), dtype=np.float32)
    for c in range(N_CORES):
        ob = res.results[c]["out"]                  # [NQ, 9, 128, 4*512]
        ob = ob.reshape(NQ, 9, 128, 4, 512).transpose(0, 3, 2, 1, 4)
        out += ob.reshape(L, D).astype(np.float32)
    return out.reshape(x.shape), res


def kernel(**inputs) -> np.ndarray:
    out, _ = run(inputs, trace=False)
    return out


# revision 38
# speedup vs baseline: 1.6013x; 1.1996x over previous
"""Trainium2 Bass kernel for GQA attention (32 q heads / 16 kv heads, head_dim
128, L=2048, D=4608) with RoPE, tanh softcap 50, causal mask, o_proj.

Strategy: tensor-parallel over heads across 8 NeuronCores. Core c computes
q-heads 4c..4c+3 and kv-heads 2c..2c+1 end-to-end; the host sums the 8 partial
[L, D] outputs (bf16 partials, f32 host accumulation).

v2 design (vs the two-phase baseline):
  - single software-pipelined pass over the 4 q-chunks of 512: causality lets
    attention for chunk nq start right after its projections (K/V history for
    chunks <= nq is already computed), so the Scalar engine's tanh+exp stream
    (~200us) hides under the PE's projection matmuls instead of serializing a
    separate attention phase
  - PV computed in [d, q] layout (lhsT = V tile, rhs = P^T tile, 512-wide
    streams) so every PE matmul streams >= 256 columns and LDWEIGHTS stays
    shadow-loaded; this also eliminates the per-128-column PE transposes of
    the attention output (o_proj consumes [d, q] directly)
  - softmax denominator accumulated on the otherwise-idle GpSimd engine
    (tensor_add over P^T tiles + partition_all_reduce broadcast), reciprocal
    on DVE, folded into the PV psum drain multiply
  - rope drains moved off the Scalar engine: DVE multiplies read the
    projection psum directly (cos/sin mul + rotate-half add)
  - wq/wk/wv resident; wo streamed per (chunk, j) to fit SBUF; x staged per
    chunk; outputs written bf16
"""

import numpy as np
import ml_dtypes

import concourse.bass as bass
import concourse.mybir as mybir
import concourse.tile as tile
from concourse import bacc, bass_isa

F32 = mybir.dt.float32
BF16 = mybir.dt.bfloat16
BF16_NP = ml_dtypes.bfloat16
AF = mybir.ActivationFunctionType

N_HEADS = 32
N_KV = 16
HEAD_DIM = 128
ROPE_THETA = 10000.0
SOFTCAP = 50.0
SCALE = 1.0 / 12.0  # 1/sqrt(144)
L = 2048
D = 4608
N_CORES = 8
QH = N_HEADS // N_CORES        # 4 local q heads
KVH = N_KV // N_CORES          # 2 local kv heads
KC = D // 128                  # 36 contraction chunks
NQ = L // 512                  # 4 l-chunks of 512
LT = L // 128                  # 16 l-tiles of 128


DEBUG_TAPS = False


def _emit(nc):
    xt_d = nc.dram_tensor("xt", [2 * NQ, 128, KC * 256], BF16, kind="ExternalInput")
    wqt_d = nc.dram_tensor("wqt", [128, KC * QH * 128], BF16, kind="ExternalInput")
    wkt_d = nc.dram_tensor("wkt", [128, KC * KVH * 128], BF16, kind="ExternalInput")
    wvt_d = nc.dram_tensor("wvt", [128, KC * KVH * 128], BF16, kind="ExternalInput")
    wot_d = nc.dram_tensor("wot", [9, 128, QH * 512], BF16, kind="ExternalInput")
    cs_d = nc.dram_tensor("cs", [2 * NQ, 128, 512], BF16, kind="ExternalInput")
    masks_d = nc.dram_tensor("masks", [128, 1280], BF16, kind="ExternalInput")
    out_d = nc.dram_tensor("out", [NQ, 9, 128, 4 * 512], BF16, kind="ExternalOutput")
    if DEBUG_TAPS:
        qt_dbg = nc.dram_tensor("qt_dbg", [QH, 128, 512], BF16, kind="ExternalOutput")
        kt_dbg = nc.dram_tensor("kt_dbg", [KVH, 128, L], BF16, kind="ExternalOutput")
        ve_dbg = nc.dram_tensor("ve_dbg", [128, LT * 256], BF16, kind="ExternalOutput")
        at_dbg = nc.dram_tensor("at_dbg", [QH, 128, 512], BF16, kind="ExternalOutput")
        rb_dbg = nc.dram_tensor("rb_dbg", [QH, 128, 512], F32, kind="ExternalOutput")
        dn_dbg = nc.dram_tensor("dn_dbg", [128, 512], F32, kind="ExternalOutput")
        pr_dbg = nc.dram_tensor("pr_dbg", [128, 512], F32, kind="ExternalOutput")
        pt_dbg = nc.dram_tensor("pt_dbg", [4, 128, 512], BF16, kind="ExternalOutput")

    with tile.TileContext(nc) as tc:
        with (
            tc.tile_pool(name="const", bufs=1) as const,
            tc.tile_pool(name="wts", bufs=1) as wts,
            tc.tile_pool(name="wo", bufs=2) as wop,
            tc.tile_pool(name="xp", bufs=2) as xp,
            tc.tile_pool(name="cs", bufs=1) as csp,
            tc.tile_pool(name="qt", bufs=2) as qtp,
            tc.tile_pool(name="persist", bufs=1) as persist,
            tc.tile_pool(name="pt", bufs=1) as ptp,
            tc.tile_pool(name="rp", bufs=1) as rpp,
            tc.tile_pool(name="tt", bufs=1) as ttp,
            tc.tile_pool(name="rb", bufs=1) as rbp,
            tc.tile_pool(name="at", bufs=3) as atp,
            tc.tile_pool(name="ob", bufs=2) as obp,
            tc.tile_pool(name="pj_psum", bufs=2, space="PSUM") as pj_psum,
            tc.tile_pool(name="sc_psum", bufs=2, space="PSUM") as sc_psum,
            tc.tile_pool(name="pv_psum", bufs=1, space="PSUM") as pv_psum,
            tc.tile_pool(name="op_psum", bufs=2, space="PSUM") as op_psum,
        ):
            # ---- persistent tensors ----
            KT = [persist.tile([128, L], BF16, tag=f"kt{g}", name=f"kt{g}")
                  for g in range(KVH)]
            VE = persist.tile([128, LT * 256], BF16, tag="ve", name="ve")
            QTS = [[None] * QH for _ in range(NQ)]
            # ---- prologue DMA: one 2D descriptor per tensor (descriptor
            # service pace is ~2us each regardless of size; count rules) ----
            def dma_x(s2):
                t = xp.tile([128, KC * 256], BF16, tag="x", name="xc")
                nc.sync.dma_start(t[:], xt_d[s2])
                return t

            ones = const.tile([128, 128], BF16, tag="ones")
            nc.vector.memset(ones[:], 1.0)
            mtile = const.tile([128, 1280], BF16, tag="masks")
            nc.sync.dma_start(mtile[:], masks_d[:])
            moff = [0, 512, 896, 1152]
            maskt = [mtile[:, moff[o]:moff[o] + 512 - o * 128] for o in range(4)]
            xc0 = dma_x(0)
            wk = wts.tile([128, KC * KVH * 128], BF16, tag="wk", name="wk")
            nc.sync.dma_start(wk[:], wkt_d[:])
            wq = wts.tile([128, KC * QH * 128], BF16, tag="wq", name="wq")
            nc.sync.dma_start(wq[:], wqt_d[:])
            wv = wts.tile([128, KC * KVH * 128], BF16, tag="wv", name="wv")
            nc.sync.dma_start(wv[:], wvt_d[:])

            def dma_cs(s2):
                t = csp.tile([128, 512], BF16, tag="cs", name="cs")
                nc.sync.dma_start(t[:], cs_d[s2])
                return t[:, 0:256], t[:, 256:512]

            x_next = [xc0]

            def rope_drain(ps, dst, cosc, sinc):
                """psum [128,256] f32 -> rotate-half rope -> dst bf16."""
                t1 = rpp.tile([128, 256], F32, tag="r1")
                nc.vector.tensor_mul(t1[:], ps[:], cosc[:])
                t2 = rpp.tile([128, 256], F32, tag="r2")
                nc.vector.tensor_mul(t2[0:64, :], ps[64:128, :], sinc[0:64, :])
                nc.vector.tensor_mul(t2[64:128, :], ps[0:64, :], sinc[64:128, :])
                nc.vector.tensor_add(dst[:], t1[:], t2[:])

            def proj_sub(s2):
                """Projections for 256-col sub-chunk s2 (K, Q, V + rope).

                Prefetches sub-chunk s2+1's x tiles (bufs=2 ring, no WAR
                wait) so projection matmuls never stall on staging DMA.
                """
                nq, half = s2 // 2, s2 % 2
                xc = x_next[0]
                if s2 + 1 < 2 * NQ:
                    x_next[0] = dma_x(s2 + 1)
                cosc, sinc = dma_cs(s2)
                cols = slice(half * 256, half * 256 + 256)
                for b in range(2):
                    mk = s2 * 2 + b
                    ps = pj_psum.tile([128, 256], F32, tag="pj")
                    for k in range(KC):
                        nc.tensor.matmul(
                            ps[:], xc[:, k * 256 + b * 128:k * 256 + b * 128 + 128],
                            wv[:, k * 256:(k + 1) * 256],
                            start=(k == 0), stop=(k == KC - 1))
                    nc.vector.tensor_copy(
                        VE[:, mk * 256:(mk + 1) * 256], ps[:])
                for g in range(KVH):
                    ps = pj_psum.tile([128, 256], F32, tag="pj")
                    for k in range(KC):
                        nc.tensor.matmul(
                            ps[:], wk[:, k * 256 + g * 128:k * 256 + g * 128 + 128],
                            xc[:, k * 256:(k + 1) * 256],
                            start=(k == 0), stop=(k == KC - 1))
                    rope_drain(ps, KT[g][:, s2 * 256:(s2 + 1) * 256],
                               cosc, sinc)
                for h in range(QH):
                    if half == 0:
                        QTS[nq][h] = qtp.tile([128, 512], BF16, tag=f"qt{h}", name=f"qt{h}")
                    qt = QTS[nq][h]
                    ps = pj_psum.tile([128, 256], F32, tag="pj")
                    for k in range(KC):
                        nc.tensor.matmul(
                            ps[:], wq[:, k * 512 + h * 128:k * 512 + h * 128 + 128],
                            xc[:, k * 256:(k + 1) * 256],
                            start=(k == 0), stop=(k == KC - 1))
                    rope_drain(ps, qt[:, cols], cosc, sinc)

            def proj_a(nq):
                proj_sub(2 * nq)

            def proj_b(nq):
                proj_sub(2 * nq + 1)

            def scores(nq, h):
                """scores -> tanh -> exp -> causal mask (affine_select)."""
                g = h // 2
                nkt = 4 * nq + 4
                hp = h % 2
                pts = []
                for mk in range(nkt):
                    o = mk - 4 * nq
                    c0 = max(0, o) * 128
                    w = 512 - c0
                    ps_s = sc_psum.tile([128, 512], F32, tag="sc")
                    nc.tensor.matmul(
                        ps_s[:, 0:w], KT[g][:, mk * 128:(mk + 1) * 128],
                        QTS[nq][h][:, c0:512])
                    tt = ttp.tile([128, 512], F32, tag="tanh")
                    nc.scalar.activation(
                        tt[:, 0:w], ps_s[:, 0:w], AF.Tanh, scale=SCALE / SOFTCAP)
                    pt = ptp.tile([128, 512], BF16, tag=f"pt{hp}_{mk}")
                    pts.append(pt)
                    nc.scalar.activation(
                        pt[:, c0:512], tt[:, 0:w], AF.Exp, scale=SOFTCAP)
                    if o >= 0:
                        nc.vector.tensor_mul(
                            pt[:, c0:512], pt[:, c0:512], maskt[o][:, 0:w])
                return pts

            def pv(nq, h, pts):
                """attn[d, q] = sum_mk V[mk]^T @ P^T[mk]; denom = ones^T P."""
                g = h // 2
                nkt = 4 * nq + 4
                hp = h % 2
                ps = pv_psum.tile([128, 512], F32, tag="pv")
                for mk in range(nkt):
                    o = mk - 4 * nq
                    c0 = max(0, o) * 128
                    nc.tensor.matmul(
                        ps[:, c0:512],
                        VE[:, mk * 256 + g * 128:mk * 256 + g * 128 + 128],
                        pts[mk][:, c0:512],
                        start=(mk == 0), stop=(mk == nkt - 1))
                at = atp.tile([128, 512], BF16, tag=f"at{h}")
                nc.vector.tensor_copy(at[:], ps[:])
                dn = pv_psum.tile([128, 512], F32, tag="dn")
                for mk in range(nkt):
                    o = mk - 4 * nq
                    c0 = max(0, o) * 128
                    nc.tensor.matmul(
                        dn[:, c0:512], ones[:], pts[mk][:, c0:512],
                        start=(mk == 0), stop=(mk == nkt - 1))
                rb = rbp.tile([128, 512], F32, tag=f"rb{hp}")
                nc.vector.reciprocal_approx_fast(rb[:], dn[:])
                return at, rb

            ATT = [[None] * QH for _ in range(NQ)]
            RB = [[None] * QH for _ in range(NQ)]

            def S(nq, h):
                RB[nq][h] = scores(nq, h)

            def V(nq, h):
                pts = RB[nq][h]
                ATT[nq][h] = pv(nq, h, pts)

            def N(nq, h):
                """Deferred normalization: attnT_norm = attnT_raw * 1/denom."""
                raw, rb = ATT[nq][h]
                at = atp.tile([128, 512], BF16, tag=f"at{h}", name=f"at{h}")
                nc.vector.tensor_mul(at[:], raw[:], rb[:])
                ATT[nq][h] = at

            def dma_wo(j):
                w = wop.tile([128, QH * 512], BF16, tag="wo", name="woj")
                nc.sync.dma_start(w[:], wot_d[j])
                return w

            def oproj(nq, j0, j1):
                """o_proj chunk nq for wo column-chunks j0..j1-1.

                wo tiles prefetched one j ahead so loads sit in front of the
                out-store DMAs in the SP queue.
                """
                wo_cur = dma_wo(j0)
                for j in range(j0, j1):
                    woj = wo_cur
                    if j + 1 < j1:
                        wo_cur = dma_wo(j + 1)
                    ob = obp.tile([128, 4 * 512], BF16, tag="ob", name="ob4")
                    for s in range(4):
                        po = op_psum.tile([128, 512], F32, tag="op")
                        for h in range(QH):
                            nc.tensor.matmul(
                                po[:], ATT[nq][h][:, s * 128:(s + 1) * 128],
                                woj[:, h * 512:(h + 1) * 512],
                                start=(h == 0), stop=(h == QH - 1))
                        nc.vector.tensor_copy(ob[:, s * 512:(s + 1) * 512], po[:])
                    nc.sync.dma_start(out_d[nq, j], ob[:])

            # ---- software-pipelined schedule ----
            # Each slot pairs scalar-heavy score work with PE-heavy projection
            # or o_proj streams so tanh/exp always hides under matmuls.
            proj_a(0); proj_b(0)
            if DEBUG_TAPS:
                for h in range(QH):
                    nc.sync.dma_start(qt_dbg[h], QTS[0][h][:])
            S(0, 0); S(0, 1)
            proj_a(1)
            V(0, 0); S(0, 2); N(0, 0)
            proj_b(1)
            V(0, 1); S(0, 3); N(0, 1)
            proj_a(2)
            V(0, 2); S(1, 0); N(0, 2)
            proj_b(2)
            V(0, 3); S(1, 1); N(0, 3)
            oproj(0, 0, 5)
            V(1, 0); S(1, 2); N(1, 0)
            oproj(0, 5, 9)
            V(1, 1); S(1, 3); N(1, 1)
            proj_a(3)
            V(1, 2); S(2, 0); N(1, 2)
            proj_b(3)
            V(1, 3); S(2, 1); N(1, 3)
            oproj(1, 0, 5)
            V(2, 0); S(2, 2); N(2, 0)
            oproj(1, 5, 9)
            V(2, 1); S(2, 3); N(2, 1)
            V(2, 2); S(3, 0); N(2, 2)
            V(2, 3); S(3, 1); N(2, 3)
            oproj(2, 0, 5)
            V(3, 0); S(3, 2); N(3, 0)
            oproj(2, 5, 9)
            V(3, 1); S(3, 3); N(3, 1)
            V(3, 2); N(3, 2)
            V(3, 3); N(3, 3)
            oproj(3, 0, 9)
            if DEBUG_TAPS:
                for g in range(KVH):
                    nc.sync.dma_start(kt_dbg[g], KT[g][:])
                nc.sync.dma_start(ve_dbg[:], VE[:])
    return nc


_CACHED_NC = {}


def build():
    if 0 not in _CACHED_NC:
        nc = bacc.Bacc("TRN2", target_bir_lowering=False, debug=False)
        _emit(nc)
        nc.compile()
        _CACHED_NC[0] = nc
    return _CACHED_NC[0]


def host_tables():
    inv_freq = 1.0 / (ROPE_THETA ** (np.arange(0, HEAD_DIM, 2, dtype=np.float32) / HEAD_DIM))
    ang = np.arange(L, dtype=np.float32)[:, None] * inv_freq[None, :]  # [L, 64]
    cos, sin = np.cos(ang), np.sin(ang)
    cosT = np.concatenate([cos.T, cos.T], axis=0).astype(BF16_NP)  # [128, L]
    sinT = np.concatenate([-sin.T, sin.T], axis=0).astype(BF16_NP)
    # packed [8, 128, 512]: per 256-col sub-chunk, cos cols then sin cols
    cs = np.empty((2 * NQ, 128, 512), BF16_NP)
    for s2 in range(2 * NQ):
        cs[s2, :, 0:256] = cosT[:, s2 * 256:(s2 + 1) * 256]
        cs[s2, :, 256:512] = sinT[:, s2 * 256:(s2 + 1) * 256]
    return np.ascontiguousarray(cs)


def host_masks():
    k = np.arange(128)[:, None]
    m = np.empty((128, 1280), BF16_NP)
    moff = [0, 512, 896, 1152]
    for o in range(4):
        q = np.arange(o * 128, 512)[None, :]
        m[:, moff[o]:moff[o] + 512 - o * 128] = (q >= k + 128 * o)
    return np.ascontiguousarray(m)


def _pack_kblocks(wT, width):
    """[KC*128, width] -> SBUF image [128, KC*width] (k-blocks along free)."""
    return np.ascontiguousarray(
        wT.reshape(KC, 128, width).transpose(1, 0, 2).reshape(128, KC * width))


def make_in_maps(x, wq, wk, wv, wo):
    cs = host_tables()
    masks = host_masks()
    xT = x.reshape(L, D).T.astype(BF16_NP)          # [D, L]
    # packed x: [8, 128, KC*256]: xb[s2, p, k*256+c] = xT[k*128+p, s2*256+c]
    xb = np.ascontiguousarray(
        xT.reshape(KC, 128, 2 * NQ, 256).transpose(2, 1, 0, 3)
        .reshape(2 * NQ, 128, KC * 256))
    in_maps = []
    for c in range(N_CORES):
        qs = slice(c * QH * 128, (c + 1) * QH * 128)
        kvs = slice(c * KVH * 128, (c + 1) * KVH * 128)
        woT = wo[:, qs].T.astype(BF16_NP)           # [512, D]
        # wo packed [9, 128, QH*512]: [j, p, h*512+c] = woT[h*128+p, j*512+c]
        wob = np.ascontiguousarray(
            woT.reshape(QH, 128, 9, 512).transpose(2, 1, 0, 3)
            .reshape(9, 128, QH * 512))
        in_maps.append({
            "xt": xb,
            "wqt": _pack_kblocks(wq[qs].T.astype(BF16_NP), QH * 128),
            "wkt": _pack_kblocks(wk[kvs].T.astype(BF16_NP), KVH * 128),
            "wvt": _pack_kblocks(wv[kvs].T.astype(BF16_NP), KVH * 128),
            "wot": wob,
            "cs": cs,
            "masks": masks,
        })
    return in_maps


def run(inputs, trace=False, trace_kwargs=None):
    from concourse.bass_utils import run_bass_kernel_spmd

    nc = build()
    x = np.asarray(inputs["x"], dtype=np.float32)
    in_maps = make_in_maps(
        x,
        np.asarray(inputs["wq"], dtype=np.float32),
        np.asarray(inputs["wk"], dtype=np.float32),
        np.asarray(inputs["wv"], dtype=np.float32),
        np.asarray(inputs["wo"], dtype=np.float32),
    )
    res = run_bass_kernel_spmd(
        nc, in_maps, core_ids=list(range(N_CORES)),
        trace=trace, **(trace_kwargs or {}))
    out = np.zeros((L, D), dtype=np.float32)
    for c in range(N_CORES):
        ob = res.results[c]["out"]                  # [NQ, 9, 128, 4*512]
        ob = ob.reshape(NQ, 9, 128, 4, 512).transpose(0, 3, 2, 1, 4)
        out += ob.reshape(L, D).astype(np.float32)
    return out.reshape(x.shape), res


def kernel(**inputs) -> np.ndarray:
    out, _ = run(inputs, trace=False)
    return out
